# revision 1
# baseline (speedup 1.0000x reference)
"""Trainium2 Bass kernel for nn_ContextEncoder (banded local attention encoder).

Reference computation (B=2, T=2048, D=512, H=8, dh=64, band half-width 32):
  xn   = LayerNorm(x) * g + b
  q    = ((xn @ Wp.T + bp) @ Wq.T + bq) / sqrt(dh)      per-head [B,T,H,dh]
  k, v = xn @ Wk.T + bk, xn @ Wv.T + bv
  s    = banded scores  (|i-j| <= 32), softmax over window
  ctx  = (a @ v_window) @ Wo.T + bo
  gate = sigmoid([x, ctx] @ Wg.T + bg)
  out  = x * (1 - gate) + ctx * gate

Sharding: sequence-parallel, 8 cores = 2 batches x 4 chunks of 512 tokens.
Each core gets its 512-token chunk plus a 32-token halo on each side
(zero-padded at sequence edges; per-core masks kill invalid positions),
computes its 512 output rows fully independently (no collectives), and the
host concatenates.

Device layout choices:
  - LayerNorm token-major (bn_stats), then DMA-xbar-transpose to feature-major
    xnT [d, tok] (bf16) which feeds every projection without further
    transposes.
  - qT/kT feature-major via weight-stationary matmuls; v token-major with a
    ones-column interleaved per head (so the attention A @ [1|V] matmul
    emits softmax denominators for free).
  - Scores computed transposed S^T[w, q] per (128-query block, head) over a
    192-wide key window; exp on ACT, multiplicative band/boundary mask on DVE
    (fp32 probabilities), A-stationary AV matmul -> ctx token-major,
    normalized by the reciprocal denominator (per-partition scalar).
  - ctx DMA-transposed to feature-major for the O/gate projections
    (token-major outputs), sigmoid on ACT, final blend elementwise.

All weights are pre-transposed/fused on host (LN gain folded into Wp/Wk/Wv,
1/sqrt(dh) folded into Wq) and shipped bf16.
"""

import numpy as np
import ml_dtypes

B, T, D = 2, 2048, 512
H, DH = 8, 64
WCTX = 32
NCORES = 8
CHUNK = 512          # tokens per core
NBLK = CHUNK // 128  # 4 query blocks per core
HALO = CHUNK + 2 * WCTX   # 576 tokens incl. halo
XROWS = 640          # x dram rows: 512 central + 32 left + 32 right + 64 pad
BF16 = ml_dtypes.bfloat16

_CACHE = {}


def _build_program(flags, stage=6, s4=5):
    """Builds the single-core Bass/Tile program (shared SPMD across 8 cores).

    flags: (bo_nonzero, bg_nonzero) -> emit optional bias adds.
    """
    import concourse.bass as bass
    import concourse.tile as tile
    import concourse.mybir as mybir
    from concourse import bacc

    f32 = mybir.dt.float32
    bf16 = mybir.dt.bfloat16
    AF = mybir.ActivationFunctionType
    ALU = mybir.AluOpType
    bo_nz, bg_nz = flags

    nc = bacc.Bacc(
        "TRN2",
        target_bir_lowering=False,
        debug=False,
        enable_asserts=False,
        num_devices=NCORES,
    )

    x_in = nc.dram_tensor("x", [XROWS, D], f32, kind="ExternalInput")
    xt_in = nc.dram_tensor("xt", [D, CHUNK], bf16, kind="ExternalInput")
    mA_in = nc.dram_tensor("mA", [128, NBLK, 128], bf16, kind="ExternalInput")
    mB_in = nc.dram_tensor("mB", [64, NBLK, 128], bf16, kind="ExternalInput")
    w_in = {
        n: nc.dram_tensor(n, [D, D], bf16, kind="ExternalInput")
        for n in ["wp", "wq", "wk", "wv", "wo", "wg1", "wg2"]
    }
    # feature-major per-partition biases, [128, 4] layout (col = d-tile)
    bql_in = nc.dram_tensor("bql", [128, 4], f32, kind="ExternalInput")  # bp_eff
    bqh_in = nc.dram_tensor("bqh", [128, 4], f32, kind="ExternalInput")  # bq/8
    bkl_in = nc.dram_tensor("bkl", [128, 4], f32, kind="ExternalInput")  # bk_eff
    bv_in = nc.dram_tensor("bv", [1, D], f32, kind="ExternalInput")      # bv_eff
    bo_in = nc.dram_tensor("bo", [1, D], f32, kind="ExternalInput")
    bg_in = nc.dram_tensor("bg", [1, D], f32, kind="ExternalInput")
    out_t = nc.dram_tensor("out", [CHUNK, D], f32, kind="ExternalOutput")

    def rep_ap(ap, axis_pos, n):
        """Insert a stride-0 dim of size n into an AP at free-dim position."""
        aps = [list(p) for p in ap.ap]
        aps.insert(axis_pos, [0, n])
        return bass.AP(tensor=ap.tensor, offset=ap.offset, ap=aps)

    with tile.TileContext(nc) as tc:
        with (
            tc.tile_pool(name="wpool", bufs=1) as wpool,
            tc.tile_pool(name="apool", bufs=1) as apool,
            tc.tile_pool(name="small", bufs=1) as small,
            tc.tile_pool(name="stats", bufs=6) as stats_pool,
            tc.tile_pool(name="attn", bufs=2) as attn_pool,
            tc.tile_pool(name="fin", bufs=3) as fin_pool,
            tc.tile_pool(name="pp", bufs=3, space="PSUM") as pp,
            tc.tile_pool(name="sp0", bufs=2, space="PSUM") as sp0,
            tc.tile_pool(name="sp1", bufs=1, space="PSUM") as sp1,
            tc.tile_pool(name="cp", bufs=2, space="PSUM") as cp,
        ):
            # ---- persistent SBUF tensors ----
            x_sb = apool.tile([128, 5, D], f32, tag="x")
            xn0 = apool.tile([128, 5, D], bf16, tag="xn0")
            xnT = apool.tile([128, 4, HALO], bf16, tag="xnT")
            qinT = apool.tile([128, 4, CHUNK], bf16, tag="qinT")
            qT = apool.tile([128, 4, CHUNK], bf16, tag="qT")
            kT = apool.tile([128, 4, HALO], bf16, tag="kT")
            v_sb = apool.tile([128, 5, 8 * 65], bf16, tag="v")
            xt_sb = apool.tile([128, 4, CHUNK], bf16, tag="xt")
            mA_sb = apool.tile([128, NBLK, 128], bf16, tag="mA")
            mB_sb = apool.tile([64, NBLK, 128], bf16, tag="mB")

            ws = {n: wpool.tile([128, 4, D], bf16, tag=n, name=n) for n in w_in}
            bql = small.tile([128, 4], f32, tag="bql")
            bqh = small.tile([128, 4], f32, tag="bqh")
            bkl = small.tile([128, 4], f32, tag="bkl")
            bv_bc = small.tile([128, D], f32, tag="bv_bc")
            eps_t = small.tile([128, 1], f32, tag="eps")

            # ---- input DMAs ----
            # x split per 128-token tile so LayerNorm starts after the first
            x_view = x_in[:].rearrange("(c p) d -> p c d", p=128)
            for t in range(5):
                nc.sync.dma_start(out=x_sb[:, t, :], in_=x_view[:, t, :])
            for n in w_in:
                nc.sync.dma_start(
                    out=ws[n][:], in_=w_in[n][:].rearrange("(c p) d -> p c d", p=128)
                )
            nc.sync.dma_start(
                out=xt_sb[:], in_=xt_in[:].rearrange("(c p) d -> p c d", p=128)
            )
            nc.sync.dma_start(out=mA_sb[:], in_=mA_in[:])
            nc.sync.dma_start(out=mB_sb[:], in_=mB_in[:])
            nc.sync.dma_start(out=bql[:], in_=bql_in[:])
            nc.sync.dma_start(out=bqh[:], in_=bqh_in[:])
            nc.sync.dma_start(out=bkl[:], in_=bkl_in[:])
            nc.gpsimd.dma_start(out=bv_bc[:], in_=bv_in[:].to_broadcast([128, D]))
            bo_bc = bg_bc = None
            if bo_nz:
                bo_bc = small.tile([128, D], f32, tag="bo_bc")
                nc.gpsimd.dma_start(out=bo_bc[:], in_=bo_in[:].to_broadcast([128, D]))
            if bg_nz:
                bg_bc = small.tile([128, D], f32, tag="bg_bc")
                nc.gpsimd.dma_start(out=bg_bc[:], in_=bg_in[:].to_broadcast([128, D]))

            ident = small.tile([128, 128], bf16, tag="ident")
            from concourse.masks import make_identity
            make_identity(nc, ident[:])

            nc.vector.memset(eps_t[:], 1e-5)
            # ones column per head in v (ones at interleaved position 65h)
            v_view = v_sb[:].rearrange("p t (h c) -> p t h c", c=65)
            nc.gpsimd.memset(v_view[:, :, :, 0:1], 1.0)

            # ---- stage 1: LayerNorm (token-major, permuted layout) ----
            for t in range(5):
                rows = 128 if t < 4 else 64
                st = stats_pool.tile([128, 6], f32, tag="st")
                mv = stats_pool.tile([128, 2], f32, tag="mv")
                rstd = stats_pool.tile([128, 1], f32, tag="rstd")
                nc.vector.bn_stats(out=st[:rows], in_=x_sb[:rows, t, :])
                nc.vector.bn_aggr(out=mv[:rows], in_=st[:rows])
                nc.scalar.activation(
                    out=rstd[:rows], in_=mv[:rows, 1:2], func=AF.Sqrt,
                    bias=eps_t[:rows], scale=1.0,
                )
                nc.vector.reciprocal(out=rstd[:rows], in_=rstd[:rows])
                nc.vector.tensor_scalar(
                    out=xn0[:rows, t, :], in0=x_sb[:rows, t, :],
                    scalar1=mv[:rows, 0:1], scalar2=rstd[:rows],
                    op0=ALU.subtract, op1=ALU.mult,
                )

            if stage >= 2:
                # ---- stage 2: transpose xn0 -> xnT via PE (halo-frame order) ----
                # x rows: [0:512] central (halo 32..544), [512:544] left halo
                # (halo 0..32), [544:576] right halo (halo 544..576)
                for j in range(4):
                    tp = pp.tile([128, 512], bf16, tag="pp", name=f"tpx{j}")
                    for t in range(4):
                        nc.tensor.transpose(
                            tp[:, 128 * t: 128 * (t + 1)],
                            xn0[:, t, 128 * j: 128 * (j + 1)],
                            ident[:],
                        )
                    nc.scalar.activation(
                        out=xnT[:, j, 32:544], in_=tp[:], func=AF.Copy
                    )
                    th = pp.tile([128, 512], bf16, tag="pp", name=f"thx{j}")
                    nc.tensor.transpose(
                        th[:, 0:64],
                        xn0[0:64, 4, 128 * j: 128 * (j + 1)],
                        ident[0:64, 0:64],
                    )
                    _base = xnT[:, j, :]
                    halo_out = bass.AP(
                        tensor=_base.tensor,
                        offset=_base.offset,
                        ap=[list(_base.ap[0]), [544, 2], [1, 32]],
                    )
                    nc.scalar.activation(
                        out=halo_out, in_=th[:, 0:64].rearrange("p (a b) -> p a b", b=32),
                        func=AF.Copy,
                    )

            if stage >= 3:
                # ---- stage 3: projections ----
                # qinT[d, q] = Wp_eff @ xnT  (central tokens only)
                for j in range(4):
                    ps = pp.tile([128, 512], f32, tag="pp")
                    for c in range(4):
                        nc.tensor.matmul(
                            ps[:], ws["wp"][:, c, 128 * j: 128 * (j + 1)],
                            xnT[:, c, 32: 32 + CHUNK],
                            start=(c == 0), stop=(c == 3),
                        )
                    nc.scalar.activation(
                        out=qinT[:, j, :], in_=ps[:], func=AF.Identity,
                        bias=bql[:, j: j + 1], scale=1.0,
                    )
                # qT[d, q] = (Wq/8) @ qinT
                for j in range(4):
                    ps = pp.tile([128, 512], f32, tag="pp")
                    for c in range(4):
                        nc.tensor.matmul(
                            ps[:], ws["wq"][:, c, 128 * j: 128 * (j + 1)],
                            qinT[:, c, :],
                            start=(c == 0), stop=(c == 3),
                        )
                    nc.scalar.activation(
                        out=qT[:, j, :], in_=ps[:], func=AF.Identity,
                        bias=bqh[:, j: j + 1], scale=1.0,
                    )
                # kT[d, w] = Wk_eff @ xnT  (all 576 halo tokens, split 512+64)
                for j in range(4):
                    ps = pp.tile([128, 512], f32, tag="pp")
                    ps2 = pp.tile([128, 512], f32, tag="pp")
                    for c in range(4):
                        nc.tensor.matmul(
                            ps[:], ws["wk"][:, c, 128 * j: 128 * (j + 1)],
                            xnT[:, c, 0:512],
                            start=(c == 0), stop=(c == 3),
                        )
                    for c in range(4):
                        nc.tensor.matmul(
                            ps2[:, 0:64], ws["wk"][:, c, 128 * j: 128 * (j + 1)],
                            xnT[:, c, 512:576],
                            start=(c == 0), stop=(c == 3),
                        )
                    nc.scalar.activation(
                        out=kT[:, j, 0:512], in_=ps[:], func=AF.Identity,
                        bias=bkl[:, j: j + 1], scale=1.0,
                    )
                    nc.scalar.activation(
                        out=kT[:, j, 512:576], in_=ps2[:, 0:64], func=AF.Identity,
                        bias=bkl[:, j: j + 1], scale=1.0,
                    )
                # v[w, d] token-major (+ bias broadcast), interleaved ones cols
                for t in range(5):
                    rows = 128 if t < 4 else 64
                    ps = pp.tile([128, 512], f32, tag="pp")
                    for c in range(4):
                        nc.tensor.matmul(
                            ps[:rows], xnT[:, c, 128 * t: 128 * t + rows],
                            ws["wv"][:, c, :],
                            start=(c == 0), stop=(c == 3),
                        )
                    nc.vector.tensor_add(
                        out=v_view[:rows, t, :, 1:65],
                        in0=ps[:rows].rearrange("p (h c) -> p h c", c=64),
                        in1=bv_bc[:rows].rearrange("p (h c) -> p h c", c=64),
                    )

            if stage >= 4:
                # gate part 1 (x @ Wg1.T) is independent of attention; compute
                # it early so only the ctx part remains on the critical tail
                g1_sb = apool.tile([128, 4, 512], f32, tag="g1")
                for qt in range(4):
                    ps = pp.tile([128, 512], f32, tag="pp")
                    for c in range(4):
                        nc.tensor.matmul(
                            ps[:], xt_sb[:, c, 128 * qt: 128 * (qt + 1)],
                            ws["wg1"][:, c, :],
                            start=(c == 0), stop=(c == 3),
                        )
                    if bg_nz:
                        nc.vector.tensor_add(
                            out=g1_sb[:, qt, :], in0=ps[:], in1=bg_bc[:]
                        )
                    else:
                        nc.scalar.activation(
                            out=g1_sb[:, qt, :], in_=ps[:], func=AF.Copy
                        )
                # HW rejects matmul operands at partition base 64, so build a
                # zero-padded per-head copy of q (other head's 64 rows = 0)
                # and contract over K=128 with the full two-head kT tile.
                q2 = apool.tile([128, 8, CHUNK], bf16, tag="q2")
                for h in range(H):
                    oh = (h % 2) * 64
                    zh = 64 - oh
                    nc.gpsimd.memset(q2[zh: zh + 64, h, :], 0.0)
                    nc.vector.tensor_copy(
                        out=q2[oh: oh + 64, h, :], in_=qT[oh: oh + 64, h // 2, :]
                    )
                # ---- stage 4: banded attention ----
                # (block, half-head-group) granularity: 1-bank PSUM tiles,
                # double-buffered so the PE never waits on exp/mask/normalize
                for b in range(NBLK):
                    ctxn_b = attn_pool.tile([128, 512], bf16, tag="ctxnb")
                    for g in range(2):
                        s0 = sp0.tile([128, 512], f32, tag="s0")
                        s1 = sp1.tile([64, 512], f32, tag="s1")
                        for hh in range(4):
                            h = 4 * g + hh
                            q_ap = q2[:, h, 128 * b: 128 * (b + 1)]
                            nc.tensor.matmul(
                                s0[:, 128 * hh: 128 * (hh + 1)],
                                kT[:, h // 2, 128 * b: 128 * b + 128],
                                q_ap, start=True, stop=True,
                            )
                            nc.tensor.matmul(
                                s1[:, 128 * hh: 128 * (hh + 1)],
                                kT[:, h // 2, 128 * b + 128: 128 * b + 192],
                                q_ap, start=True, stop=True,
                            )
                        a0 = attn_pool.tile([128, 4, 128], bf16, tag="a0")
                        a1 = attn_pool.tile([64, 4, 128], bf16, tag="a1")
                        nc.scalar.activation(
                            out=a0[:].rearrange("p h r -> p (h r)"), in_=s0[:],
                            func=AF.Exp,
                        )
                        nc.scalar.activation(
                            out=a1[:].rearrange("p h r -> p (h r)"), in_=s1[:],
                            func=AF.Exp,
                        )
                        nc.vector.tensor_mul(
                            out=a0[:], in0=a0[:], in1=rep_ap(mA_sb[:, b, :], 1, 4)
                        )
                        nc.vector.tensor_mul(
                            out=a1[:], in0=a1[:], in1=rep_ap(mB_sb[:, b, :], 1, 4)
                        )
                        # AV: ctx_aug[q, 65hh:65hh+65] = A_h @ [1 | V_h]
                        cps = cp.tile([128, 260], f32, tag="cp")
                        for hh in range(4):
                            h = 4 * g + hh
                            col = 65 * hh
                            nc.tensor.matmul(
                                cps[:, col: col + 65], a0[:, hh, :],
                                v_sb[:, b, 65 * h: 65 * (h + 1)],
                                start=True, stop=False,
                            )
                            nc.tensor.matmul(
                                cps[:, col: col + 65], a1[:, hh, :],
                                v_sb[0:64, b + 1, 65 * h: 65 * (h + 1)],
                                start=False, stop=True,
                            )
                        rd = stats_pool.tile([128, 4], f32, tag="rd")
                        nc.vector.reciprocal(
                            out=rd[:],
                            in_=cps[:].rearrange("p (h c) -> p h c", c=65)[:, :, 0],
                        )
                        for hh in range(4):
                            h = 4 * g + hh
                            src_ap = cps[:, 65 * hh + 1: 65 * hh + 65]
                            dst_ap = ctxn_b[:, 64 * h: 64 * (h + 1)]
                            if hh < 2:
                                nc.scalar.activation(
                                    out=dst_ap, in_=src_ap, func=AF.Copy,
                                    scale=rd[:, hh: hh + 1],
                                )
                            else:
                                nc.vector.tensor_scalar_mul(
                                    out=dst_ap, in0=src_ap, scalar1=rd[:, hh: hh + 1],
                                )

                    if stage < 5:
                        continue
                    # ---- per-block epilogue: transpose ctx, O-proj, gate,
                    # blend, store -- pipelines with the next block's attention
                    tp = pp.tile([128, 512], bf16, tag="pp", name=f"tpc{b}")
                    for j in range(4):
                        nc.tensor.transpose(
                            tp[:, 128 * j: 128 * (j + 1)],
                            ctxn_b[:, 128 * j: 128 * (j + 1)],
                            ident[:],
                        )
                    ctxTb = fin_pool.tile([128, 4, 128], bf16, tag="ctxTb")
                    nc.scalar.activation(
                        out=ctxTb[:].rearrange("p c q -> p (c q)"), in_=tp[:],
                        func=AF.Copy,
                    )
                    if stage < 6:
                        continue
                    ops = pp.tile([128, 512], f32, tag="pp")
                    gps = pp.tile([128, 512], f32, tag="pp")
                    for c in range(4):
                        nc.tensor.matmul(
                            ops[:], ctxTb[:, c, :], ws["wo"][:, c, :],
                            start=(c == 0), stop=(c == 3),
                        )
                    for c in range(4):
                        nc.tensor.matmul(
                            gps[:], ctxTb[:, c, :], ws["wg2"][:, c, :],
                            start=(c == 0), stop=(c == 3),
                        )
                    gate = fin_pool.tile([128, 512], f32, tag="gate")
                    diff = fin_pool.tile([128, 512], f32, tag="diff")
                    outs = fin_pool.tile([128, 512], f32, tag="outs")
                    # gate_pre = (x @ Wg1.T, hoisted) + (ctx @ Wg2o.T)
                    nc.vector.tensor_add(out=gps[:], in0=gps[:], in1=g1_sb[:, b, :])
                    nc.scalar.activation(out=gate[:], in_=gps[:], func=AF.Sigmoid)
                    if bo_nz:
                        nc.vector.tensor_add(out=ops[:], in0=ops[:], in1=bo_bc[:])
                    # out = x + gate * (o - x)
                    nc.vector.tensor_sub(out=diff[:], in0=ops[:], in1=x_sb[:, b, :])
                    nc.gpsimd.tensor_mul(out=diff[:], in0=diff[:], in1=gate[:])
                    nc.vector.tensor_add(out=outs[:], in0=diff[:], in1=x_sb[:, b, :])
                    nc.sync.dma_start(
                        out=out_t[:].rearrange("(c p) d -> p c d", p=128)[:, b, :],
                        in_=outs[:],
                    )
            if stage < 6:
                for qt in range(4):
                    nc.sync.dma_start(
                        out=out_t[:].rearrange("(c p) d -> p c d", p=128)[:, qt, :],
                        in_=x_sb[:, qt, :],
                    )
    nc.compile()
    return nc


def _host_prep(inputs):
    """Fold LN gain/bias + scale into weights, build per-core input maps."""
    x = np.asarray(inputs["token_embeds"], np.float32)
    g = np.asarray(inputs["ln_g"], np.float32)
    lb = np.asarray(inputs["ln_b"], np.float32)
    Wp = np.asarray(inputs["Wp"], np.float32)
    Wq = np.asarray(inputs["Wq"], np.float32)
    Wk = np.asarray(inputs["Wk"], np.float32)
    Wv = np.asarray(inputs["Wv"], np.float32)
    Wo = np.asarray(inputs["Wo"], np.float32)
    Wg = np.asarray(inputs["Wg"], np.float32)
    bp = np.asarray(inputs["bp"], np.float32)
    bq = np.asarray(inputs["bq"], np.float32)
    bk = np.asarray(inputs["bk"], np.float32)
    bv = np.asarray(inputs["bv"], np.float32)
    bo = np.asarray(inputs["bo"], np.float32)
    bg = np.asarray(inputs["bg"], np.float32)

    scale = 1.0 / np.sqrt(np.float32(DH))
    wp = np.ascontiguousarray((Wp * g[None, :]).T).astype(BF16)
    wq = np.ascontiguousarray((Wq * scale).T).astype(BF16)
    wk = np.ascontiguousarray((Wk * g[None, :]).T).astype(BF16)
    wv = np.ascontiguousarray((Wv * g[None, :]).T).astype(BF16)
    wo = np.ascontiguousarray(Wo.T).astype(BF16)
    wg1 = np.ascontiguousarray(Wg[:, :D].T).astype(BF16)
    # reference gates on ctx AFTER the O-projection; fold Wo into Wg2 so the
    # gate matmul can consume pre-projection ctx directly
    wg2 = np.ascontiguousarray((Wg[:, D:] @ Wo).T).astype(BF16)
    bp_eff = Wp @ lb + bp
    bq_eff = bq * scale
    bk_eff = Wk @ lb + bk
    bv_eff = (Wv @ lb + bv).reshape(1, D)
    bql = np.ascontiguousarray(bp_eff.reshape(4, 128).T).astype(np.float32)
    bqh = np.ascontiguousarray(bq_eff.reshape(4, 128).T).astype(np.float32)
    bkl = np.ascontiguousarray(bk_eff.reshape(4, 128).T).astype(np.float32)
    bg_eff = Wg[:, D:] @ bo + bg  # gate bias picks up Wg2 @ bo from the fold
    flags = (bool(np.any(bo != 0)), bool(np.any(bg_eff != 0)))

    in_maps = []
    for core in range(NCORES):
        bi, ci = core // 4, core % 4
        s = ci * CHUNK
        xr = np.zeros((XROWS, D), np.float32)
        xr[0:CHUNK] = x[bi, s: s + CHUNK]
        if s - WCTX >= 0:
            xr[CHUNK: CHUNK + WCTX] = x[bi, s - WCTX: s]
        if s + CHUNK + WCTX <= T:
            xr[CHUNK + WCTX: CHUNK + 2 * WCTX] = x[bi, s + CHUNK: s + CHUNK + WCTX]
        xt = np.ascontiguousarray(x[bi, s: s + CHUNK].T).astype(BF16)

        # mask[b, rr, cc]: query r=128b+rr (local), key halo pos j=128b+cc
        rr = np.arange(128)[:, None]
        cc = np.arange(192)[None, :]
        m = np.zeros((NBLK, 128, 192), np.float32)
        for qb in range(NBLK):
            band = (cc - rr >= 0) & (cc - rr <= 2 * WCTX)
            gkey = s + 128 * qb + cc - WCTX + 0 * rr
            m[qb] = (band & (gkey >= 0) & (gkey < T)).astype(np.float32)
        mA = np.ascontiguousarray(m[:, :, :128].transpose(2, 0, 1)).astype(BF16)
        mB = np.ascontiguousarray(m[:, :, 128:].transpose(2, 0, 1)).astype(BF16)

        in_maps.append({
            "x": xr, "xt": xt, "mA": mA, "mB": mB,
            "wp": wp, "wq": wq, "wk": wk, "wv": wv, "wo": wo,
            "wg1": wg1, "wg2": wg2,
            "bql": bql, "bqh": bqh, "bkl": bkl,
            "bv": bv_eff.astype(np.float32),
            "bo": bo.reshape(1, D), "bg": bg_eff.reshape(1, D),
        })
    return in_maps, flags


def _run(inputs, trace=False):
    from concourse.bass_utils import run_bass_kernel_spmd

    in_maps, flags = _host_prep(inputs)
    if flags not in _CACHE:
        _CACHE[flags] = _build_program(flags)
    nc = _CACHE[flags]
    res = run_bass_kernel_spmd(nc, in_maps, list(range(NCORES)), trace=trace)
    out = np.zeros((B, T, D), np.float32)
    for core in range(NCORES):
        bi, ci = core // 4, core % 4
        out[bi, ci * CHUNK: (ci + 1) * CHUNK] = res.results[core]["out"]
    return out, res


def kernel(**inputs):
    out, _ = _run(inputs, trace=False)
    return out



# revision 7
# speedup vs baseline: 1.0293x; 1.0293x over previous
"""Trainium2 Bass kernel for nn_ContextEncoder (banded local attention encoder).

Reference computation (B=2, T=2048, D=512, H=8, dh=64, band half-width 32):
  xn   = LayerNorm(x) * g + b
  q    = ((xn @ Wp.T + bp) @ Wq.T + bq) / sqrt(dh)      per-head [B,T,H,dh]
  k, v = xn @ Wk.T + bk, xn @ Wv.T + bv
  s    = banded scores  (|i-j| <= 32), softmax over window
  ctx  = (a @ v_window) @ Wo.T + bo
  gate = sigmoid([x, ctx] @ Wg.T + bg)
  out  = x * (1 - gate) + ctx * gate

Sharding: sequence-parallel, 8 cores = 2 batches x 4 chunks of 512 tokens.
Each core gets its 512-token chunk plus a 32-token halo on each side
(zero-padded at sequence edges; per-core masks kill invalid positions),
computes its 512 output rows fully independently (no collectives), and the
host concatenates.

Algebraic folds done on host:
  - Wp folded into Wq:  q = xn @ (Wq Wp).T * s  (+ folded bias) -- removes a
    full DxD projection on device.
  - k-bias dropped entirely: a per-feature constant added to every key
    shifts each query's scores uniformly, which softmax cancels.
  - v-bias folded into bo/bg: after normalization ctx picks up exactly +bv,
    so bo_eff = Wo@bv + bo and bg_eff += Wg2@Wo@bv.
  - LN gain/bias folded into the q/k/v/g weights & biases as usual.

Device pipeline (per core):
  - LayerNorm token-major (bn_stats), PE-transpose to feature-major xnT.
  - q written straight into the zero-padded pair layout q2 (head-even in
    partitions 0-63, head-odd in 64-127) so scores batch head PAIRS with a
    single kT stationary (N=256 moving).
  - Scores S^T[w, q] per (block, head-pair); exp on ACT; multiplicative
    band/boundary mask on DVE.
  - AV with V stationary -> ctx directly FEATURE-major (no transposes).
    Head outputs are interleaved into feature tiles via a host-side
    permutation of Wv/Wo/Wg2 so each (block, head-group) lands in aligned
    PE quadrants.  Softmax denominators: ones-stationary matmul over the
    probabilities broadcasts den to all partitions; one DVE divide
    normalizes straight into the O-projection operand layout.
  - O/gate projections token-major, sigmoid on ACT, blend on DVE in column
    halves so sigmoid/mult/store pipeline; bf16 store (host upcasts).
"""

import numpy as np
import ml_dtypes

B, T, D = 2, 2048, 512
H, DH = 8, 64
WCTX = 32
NCORES = 8
CHUNK = 512          # tokens per core
NBLK = CHUNK // 128  # 4 query blocks per core
HALO = CHUNK + 2 * WCTX   # 576 tokens incl. halo
XROWS = 640          # x dram rows: 512 central + 32 left + 32 right + 64 pad
BF16 = ml_dtypes.bfloat16

_CACHE = {}


def _build_program(flags):
    """Builds the single-core Bass/Tile program (shared SPMD across 8 cores).

    flags: (bq_nz, bo_nz, bg_nz) -> emit optional bias adds.
    """
    import concourse.bass as bass
    import concourse.tile as tile
    import concourse.mybir as mybir
    from concourse import bacc

    f32 = mybir.dt.float32
    bf16 = mybir.dt.bfloat16
    AF = mybir.ActivationFunctionType
    ALU = mybir.AluOpType
    bq_nz, bo_nz, bg_nz = flags

    nc = bacc.Bacc(
        "TRN2",
        target_bir_lowering=False,
        debug=False,
        enable_asserts=False,
        num_devices=NCORES,
    )

    x_in = nc.dram_tensor("x", [XROWS, D], f32, kind="ExternalInput")
    xt_in = nc.dram_tensor("xt", [D, CHUNK], bf16, kind="ExternalInput")
    mA_in = nc.dram_tensor("mA", [128, NBLK, 128], bf16, kind="ExternalInput")
    mB_in = nc.dram_tensor("mB", [64, NBLK, 128], bf16, kind="ExternalInput")
    w_in = {
        n: nc.dram_tensor(n, [D, D], bf16, kind="ExternalInput")
        for n in ["wq", "wk", "wv", "wo", "wg1", "wg2"]
    }
    bqh_in = nc.dram_tensor("bqh", [128, 4], f32, kind="ExternalInput")
    bo_in = nc.dram_tensor("bo", [1, D], f32, kind="ExternalInput")
    bg_in = nc.dram_tensor("bg", [1, D], f32, kind="ExternalInput")
    out_t = nc.dram_tensor("out", [CHUNK, D], bf16, kind="ExternalOutput")

    def rep_ap(ap, axis_pos, n):
        """Insert a stride-0 dim of size n into an AP at free-dim position."""
        aps = [list(p) for p in ap.ap]
        aps.insert(axis_pos, [0, n])
        return bass.AP(tensor=ap.tensor, offset=ap.offset, ap=aps)

    with tile.TileContext(nc) as tc:
        with (
            tc.tile_pool(name="wpool", bufs=1) as wpool,
            tc.tile_pool(name="apool", bufs=1) as apool,
            tc.tile_pool(name="small", bufs=1) as small,
            tc.tile_pool(name="stats", bufs=6) as stats_pool,
            tc.tile_pool(name="attn", bufs=2) as attn_pool,
            tc.tile_pool(name="fin", bufs=3) as fin_pool,
            tc.tile_pool(name="pp", bufs=2, space="PSUM") as pp,
            tc.tile_pool(name="sp0", bufs=2, space="PSUM") as sp0,
            tc.tile_pool(name="sp1", bufs=1, space="PSUM") as sp1,
            tc.tile_pool(name="cp", bufs=2, space="PSUM") as cp,
            tc.tile_pool(name="bcp", bufs=1, space="PSUM") as bcp,
        ):
            # ---- persistent SBUF tensors ----
            x_sb = apool.tile([128, 5, D], f32, tag="x")
            xn0 = apool.tile([128, 5, D], bf16, tag="xn0")
            xnT = apool.tile([128, 4, HALO], bf16, tag="xnT")
            q2 = apool.tile([128, 4, 2, CHUNK], bf16, tag="q2")
            kT = apool.tile([128, 4, HALO], bf16, tag="kT")
            v_sb = apool.tile([128, 5, D], bf16, tag="v")
            xt_sb = apool.tile([128, 4, CHUNK], bf16, tag="xt")
            mA_sb = apool.tile([128, NBLK, 128], bf16, tag="mA")
            mB_sb = apool.tile([64, NBLK, 128], bf16, tag="mB")
            g1_sb = apool.tile([128, 4, D], f32, tag="g1")
            ctxT = apool.tile([128, 4, NBLK, 128], bf16, tag="ctxT")

            ws = {n: wpool.tile([128, 4, D], bf16, tag=n, name=n) for n in w_in}
            eps_t = small.tile([128, 1], f32, tag="eps")
            ones_sb = small.tile([128, 128], bf16, tag="ones")
            ident = small.tile([128, 128], bf16, tag="ident")

            # ---- input DMAs ----
            # x split per 128-token tile and per d-half across the two HW
            # queues (sync + scalar) so more DMA engines work on it at once.
            x_view = x_in[:].rearrange("(c p) d -> p c d", p=128)
            for t in range(5):
                nc.sync.dma_start(out=x_sb[:, t, 0:256], in_=x_view[:, t, 0:256])
                nc.scalar.dma_start(
                    out=x_sb[:, t, 256:512], in_=x_view[:, t, 256:512]
                )

            def wdma(eng, name):
                eng.dma_start(
                    out=ws[name][:],
                    in_=w_in[name][:].rearrange("(c p) d -> p c d", p=128),
                )

            wdma(nc.sync, "wq")
            wdma(nc.scalar, "wk")
            wdma(nc.sync, "wv")
            nc.scalar.dma_start(out=mA_sb[:], in_=mA_in[:])
            nc.scalar.dma_start(out=mB_sb[:], in_=mB_in[:])
            nc.sync.dma_start(
                out=xt_sb[:], in_=xt_in[:].rearrange("(c p) d -> p c d", p=128)
            )
            wdma(nc.scalar, "wg1")
            wdma(nc.sync, "wo")
            wdma(nc.scalar, "wg2")

            bqh = bo_bc = bg_bc = None
            if bq_nz:
                bqh = small.tile([128, 4], f32, tag="bqh")
                nc.gpsimd.dma_start(out=bqh[:], in_=bqh_in[:])
            if bo_nz:
                bo_bc = small.tile([128, D], f32, tag="bo_bc")
                nc.gpsimd.dma_start(out=bo_bc[:], in_=bo_in[:].to_broadcast([128, D]))
            if bg_nz:
                bg_bc = small.tile([128, D], f32, tag="bg_bc")
                nc.gpsimd.dma_start(out=bg_bc[:], in_=bg_in[:].to_broadcast([128, D]))

            from concourse.masks import make_identity
            make_identity(nc, ident[:])
            nc.gpsimd.memset(ones_sb[:], 1.0)
            nc.vector.memset(eps_t[:], 1e-5)
            # zero the dead half of each q2 (pair, evenodd) slot once
            for p in range(4):
                nc.gpsimd.memset(q2[64:128, p, 0, :], 0.0)
                nc.gpsimd.memset(q2[0:64, p, 1, :], 0.0)

            # ---- stage 1: LayerNorm (token-major) ----
            for t in range(5):
                rows = 128 if t < 4 else 64
                st = stats_pool.tile([128, 6], f32, tag="st")
                mv = stats_pool.tile([128, 2], f32, tag="mv")
                rstd = stats_pool.tile([128, 1], f32, tag="rstd")
                nc.vector.bn_stats(out=st[:rows], in_=x_sb[:rows, t, :])
                nc.vector.bn_aggr(out=mv[:rows], in_=st[:rows])
                nc.scalar.activation(
                    out=rstd[:rows], in_=mv[:rows, 1:2], func=AF.Sqrt,
                    bias=eps_t[:rows], scale=1.0,
                )
                nc.vector.reciprocal(out=rstd[:rows], in_=rstd[:rows])
                nc.vector.tensor_scalar(
                    out=xn0[:rows, t, :], in0=x_sb[:rows, t, :],
                    scalar1=mv[:rows, 0:1], scalar2=rstd[:rows],
                    op0=ALU.subtract, op1=ALU.mult,
                )

            # ---- stage 2: transpose xn0 -> xnT via PE (halo-frame order) ----
            # x rows: [0:512] central (halo 32..544), [512:544] left halo
            # (halo 0..32), [544:576] right halo (halo 544..576)
            for j in range(4):
                tp = pp.tile([128, 512], bf16, tag="pp", name=f"tpx{j}")
                for t in range(4):
                    nc.tensor.transpose(
                        tp[:, 128 * t: 128 * (t + 1)],
                        xn0[:, t, 128 * j: 128 * (j + 1)],
                        ident[:],
                    )
                nc.scalar.activation(
                    out=xnT[:, j, 32:544], in_=tp[:], func=AF.Copy
                )
                th = pp.tile([128, 512], bf16, tag="pp", name=f"thx{j}")
                nc.tensor.transpose(
                    th[:, 0:64],
                    xn0[0:64, 4, 128 * j: 128 * (j + 1)],
                    ident[0:64, 0:64],
                )
                _base = xnT[:, j, :]
                halo_out = bass.AP(
                    tensor=_base.tensor,
                    offset=_base.offset,
                    ap=[list(_base.ap[0]), [544, 2], [1, 32]],
                )
                nc.vector.tensor_copy(
                    out=halo_out, in_=th[:, 0:64].rearrange("p (a b) -> p a b", b=32)
                )

            # ---- stage 3: projections ----
            # q: folded single projection, written into the zero-padded pair
            # layout (head-even rows -> partitions 0-63, head-odd -> 64-127)
            for j in range(4):
                ps = pp.tile([128, 512], f32, tag="pp")
                for c in range(4):
                    nc.tensor.matmul(
                        ps[:], ws["wq"][:, c, 128 * j: 128 * (j + 1)],
                        xnT[:, c, 32: 32 + CHUNK],
                        start=(c == 0), stop=(c == 3),
                    )
                if bq_nz:
                    nc.vector.tensor_scalar(
                        out=q2[0:64, j, 0, :], in0=ps[0:64],
                        scalar1=bqh[0:64, j: j + 1], scalar2=None, op0=ALU.add,
                    )
                    nc.vector.tensor_scalar(
                        out=q2[64:128, j, 1, :], in0=ps[64:128],
                        scalar1=bqh[64:128, j: j + 1], scalar2=None, op0=ALU.add,
                    )
                else:
                    nc.vector.tensor_copy(out=q2[0:64, j, 0, :], in_=ps[0:64])
                    nc.vector.tensor_copy(out=q2[64:128, j, 1, :], in_=ps[64:128])
            # kT[d, w] = Wk_eff @ xnT  (all 576 halo tokens; k-bias dropped)
            for j in range(4):
                ps = pp.tile([128, 512], f32, tag="pp")
                ps2 = pp.tile([128, 512], f32, tag="pp")
                for c in range(4):
                    nc.tensor.matmul(
                        ps[:], ws["wk"][:, c, 128 * j: 128 * (j + 1)],
                        xnT[:, c, 0:512],
                        start=(c == 0), stop=(c == 3),
                    )
                for c in range(4):
                    nc.tensor.matmul(
                        ps2[:, 0:64], ws["wk"][:, c, 128 * j: 128 * (j + 1)],
                        xnT[:, c, 512:576],
                        start=(c == 0), stop=(c == 3),
                    )
                nc.scalar.activation(out=kT[:, j, 0:512], in_=ps[:], func=AF.Copy)
                nc.scalar.activation(
                    out=kT[:, j, 512:576], in_=ps2[:, 0:64], func=AF.Copy
                )
            # v token-major (feature-permuted Wv; no bias -- folded into bo/bg)
            for t in range(5):
                rows = 128 if t < 4 else 64
                ps = pp.tile([128, 512], f32, tag="pp")
                for c in range(4):
                    nc.tensor.matmul(
                        ps[:rows], xnT[:, c, 128 * t: 128 * t + rows],
                        ws["wv"][:, c, :],
                        start=(c == 0), stop=(c == 3),
                    )
                if t % 2 == 0:
                    nc.vector.tensor_copy(out=v_sb[:rows, t, :], in_=ps[:rows])
                else:
                    nc.scalar.activation(
                        out=v_sb[:rows, t, :], in_=ps[:rows], func=AF.Copy
                    )
            # ---- stage 4: banded attention, feature-major AV ----
            for b in range(NBLK):
                # gate part 1 (x @ Wg1.T) for this block -- fills the PE
                # while the previous block's softmax chain drains
                g1ps = pp.tile([128, 512], f32, tag="pp")
                for c in range(4):
                    nc.tensor.matmul(
                        g1ps[:], xt_sb[:, c, 128 * b: 128 * (b + 1)],
                        ws["wg1"][:, c, :],
                        start=(c == 0), stop=(c == 3),
                    )
                if bg_nz:
                    nc.vector.tensor_add(
                        out=g1_sb[:, b, :], in0=g1ps[:], in1=bg_bc[:]
                    )
                else:
                    nc.vector.tensor_copy(out=g1_sb[:, b, :], in_=g1ps[:])
                cps = cp.tile([128, 4, 128], f32, tag="cps")
                for g in range(2):
                    s0 = sp0.tile([128, 2, 256], f32, tag="s0")
                    s1 = sp1.tile([64, 2, 256], f32, tag="s1")
                    for pr in range(2):
                        p = 2 * g + pr
                        q_ap = q2[:, p, :, 128 * b: 128 * (b + 1)]
                        nc.tensor.matmul(
                            s0[:, pr, :],
                            kT[:, p, 128 * b: 128 * b + 128],
                            q_ap, start=True, stop=True,
                        )
                        nc.tensor.matmul(
                            s1[:, pr, :],
                            kT[:, p, 128 * b + 128: 128 * b + 192],
                            q_ap, start=True, stop=True,
                        )
                    a0 = attn_pool.tile([128, 2, 256], bf16, tag="a0")
                    a1 = attn_pool.tile([64, 2, 256], bf16, tag="a1")
                    nc.scalar.activation(
                        out=a0[:].rearrange("p a b -> p (a b)"),
                        in_=s0[:].rearrange("p a b -> p (a b)"), func=AF.Exp,
                    )
                    nc.scalar.activation(
                        out=a1[:].rearrange("p a b -> p (a b)"),
                        in_=s1[:].rearrange("p a b -> p (a b)"), func=AF.Exp,
                    )
                    # multiplicative band/boundary mask (same for every head)
                    mval = rep_ap(rep_ap(mA_sb[:, b, :], 1, 2), 1, 2)
                    a0v = a0[:].rearrange("p a (e q) -> p a e q", q=128)
                    nc.vector.tensor_mul(out=a0v, in0=a0v, in1=mval)
                    mvalB = rep_ap(rep_ap(mB_sb[:, b, :], 1, 2), 1, 2)
                    a1v = a1[:].rearrange("p a (e q) -> p a e q", q=128)
                    eng1 = nc.vector if g == 0 else nc.gpsimd
                    eng1.tensor_mul(out=a1v, in0=a1v, in1=mvalB)
                    # denominators, broadcast to every partition by an
                    # all-ones stationary
                    bc = bcp.tile([128, 512], f32, tag="bc")
                    nc.tensor.matmul(
                        bc[:], ones_sb[:],
                        a0[:].rearrange("p a b -> p (a b)"),
                        start=True, stop=False,
                    )
                    nc.tensor.matmul(
                        bc[:], ones_sb[0:64, :],
                        a1[:].rearrange("p a b -> p (a b)"),
                        start=False, stop=True,
                    )
                    # AV: V stationary -> ctx feature-major (head h lands in
                    # feature tile h%4, partition half h//4 = g)
                    for hh in range(4):
                        po = 64 * g
                        nc.tensor.matmul(
                            cps[po: po + 64, hh, :],
                            v_sb[:, b, 128 * hh + po: 128 * hh + po + 64],
                            a0[:, hh >> 1, 128 * (hh & 1): 128 * (hh & 1) + 128],
                            start=True, stop=False,
                        )
                        nc.tensor.matmul(
                            cps[po: po + 64, hh, :],
                            v_sb[0:64, b + 1, 128 * hh + po: 128 * hh + po + 64],
                            a1[:, hh >> 1, 128 * (hh & 1): 128 * (hh & 1) + 128],
                            start=False, stop=True,
                        )
                    # normalize + emit O-proj operand layout (recip + mult;
                    # walrus rejects a TensorTensor divide)
                    po = 64 * g
                    rbc = attn_pool.tile([64, 512], f32, tag="rbc")
                    nc.vector.reciprocal_approx_fast(
                        out=rbc[:], in_=bc[po: po + 64, :]
                    )
                    nc.vector.tensor_mul(
                        out=ctxT[po: po + 64, :, b, :],
                        in0=cps[po: po + 64, :, :],
                        in1=rbc[:].rearrange("p (a q) -> p a q", q=128),
                    )

                # ---- per-block epilogue: O-proj, gate, blend, store ----
                ops = pp.tile([128, 512], f32, tag="pp")
                gps = pp.tile([128, 512], f32, tag="pp")
                for c in range(4):
                    nc.tensor.matmul(
                        ops[:], ctxT[:, c, b, :], ws["wo"][:, c, :],
                        start=(c == 0), stop=(c == 3),
                    )
                for c in range(4):
                    nc.tensor.matmul(
                        gps[:], ctxT[:, c, b, :], ws["wg2"][:, c, :],
                        start=(c == 0), stop=(c == 3),
                    )
                gpre = fin_pool.tile([128, 512], f32, tag="gpre")
                diff = fin_pool.tile([128, 512], f32, tag="diff")
                gate = fin_pool.tile([128, 512], f32, tag="gate")
                outs = fin_pool.tile([128, 512], bf16, tag="outs")
                nc.vector.tensor_add(out=gpre[:], in0=gps[:], in1=g1_sb[:, b, :])
                if bo_nz:
                    nc.vector.tensor_add(out=ops[:], in0=ops[:], in1=bo_bc[:])
                osrc = ops
                # out = x + gate * (o - x), pipelined in column halves
                nc.vector.tensor_sub(out=diff[:], in0=osrc[:], in1=x_sb[:, b, :])
                for hfs in range(2):
                    hs = slice(256 * hfs, 256 * (hfs + 1))
                    nc.scalar.activation(
                        out=gate[:, hs], in_=gpre[:, hs], func=AF.Sigmoid
                    )
                    nc.vector.tensor_mul(
                        out=diff[:, hs], in0=diff[:, hs], in1=gate[:, hs]
                    )
                    nc.vector.tensor_add(
                        out=outs[:, hs], in0=diff[:, hs], in1=x_sb[:, b, hs]
                    )
                    nc.sync.dma_start(
                        out=out_t[:].rearrange("(c p) d -> p c d", p=128)[:, b, hs],
                        in_=outs[:, hs],
                    )
    nc.compile()
    return nc


def _host_prep(inputs):
    """Fold LN gain/bias + scale + Wp + bv into weights, build per-core maps."""
    x = np.asarray(inputs["token_embeds"], np.float32)
    g = np.asarray(inputs["ln_g"], np.float32)
    lb = np.asarray(inputs["ln_b"], np.float32)
    Wp = np.asarray(inputs["Wp"], np.float32)
    Wq = np.asarray(inputs["Wq"], np.float32)
    Wk = np.asarray(inputs["Wk"], np.float32)
    Wv = np.asarray(inputs["Wv"], np.float32)
    Wo = np.asarray(inputs["Wo"], np.float32)
    Wg = np.asarray(inputs["Wg"], np.float32)
    bp = np.asarray(inputs["bp"], np.float32)
    bq = np.asarray(inputs["bq"], np.float32)
    bk = np.asarray(inputs["bk"], np.float32)  # noqa: F841  (cancels in softmax)
    bv = np.asarray(inputs["bv"], np.float32)
    bo = np.asarray(inputs["bo"], np.float32)
    bg = np.asarray(inputs["bg"], np.float32)

    scale = 1.0 / np.sqrt(np.float32(DH))
    # feature permutation for ctx: head h features -> tile h%4, half h//4
    perm = np.zeros(D, np.int64)
    for h in range(H):
        c, gg = h % 4, h // 4
        perm[128 * c + 64 * gg: 128 * c + 64 * gg + 64] = np.arange(
            64 * h, 64 * h + 64
        )

    Wpq = (Wq @ Wp) * scale                       # folded q projection
    wq = np.ascontiguousarray((Wpq * g[None, :]).T).astype(BF16)
    wk = np.ascontiguousarray((Wk * g[None, :]).T).astype(BF16)
    wv_p = (Wv * g[None, :])[perm, :]             # permuted output features
    wv = np.ascontiguousarray(wv_p.T).astype(BF16)
    wo = np.ascontiguousarray(Wo[:, perm].T).astype(BF16)
    wg1 = np.ascontiguousarray(Wg[:, :D].T).astype(BF16)
    # reference gates on ctx AFTER the O-projection; fold Wo into Wg2 so the
    # gate matmul can consume pre-projection (permuted) ctx directly
    Wg2o = Wg[:, D:] @ Wo
    wg2 = np.ascontiguousarray(Wg2o[:, perm].T).astype(BF16)

    bq_eff = (Wq @ (Wp @ lb + bp) + bq) * scale
    bv_eff = Wv @ lb + bv
    # device ctx omits the v-bias; it re-enters as a constant through both
    # the O-projection and the folded gate projection
    bo_eff = Wo @ bv_eff + bo
    bg_eff = Wg[:, D:] @ bo_eff + bg

    bqh = np.ascontiguousarray(bq_eff.reshape(4, 128).T).astype(np.float32)
    flags = (
        bool(np.any(bq_eff != 0)),
        bool(np.any(bo_eff != 0)),
        bool(np.any(bg_eff != 0)),
    )

    in_maps = []
    for core in range(NCORES):
        bi, ci = core // 4, core % 4
        s = ci * CHUNK
        xr = np.zeros((XROWS, D), np.float32)
        xr[0:CHUNK] = x[bi, s: s + CHUNK]
        if s - WCTX >= 0:
            xr[CHUNK: CHUNK + WCTX] = x[bi, s - WCTX: s]
        if s + CHUNK + WCTX <= T:
            xr[CHUNK + WCTX: CHUNK + 2 * WCTX] = x[bi, s + CHUNK: s + CHUNK + WCTX]
        xt = np.ascontiguousarray(x[bi, s: s + CHUNK].T).astype(BF16)

        # mask[b, rr, cc]: query r=128b+rr (local), key halo pos j=128b+cc
        rr = np.arange(128)[:, None]
        cc = np.arange(192)[None, :]
        m = np.zeros((NBLK, 128, 192), np.float32)
        for qb in range(NBLK):
            band = (cc - rr >= 0) & (cc - rr <= 2 * WCTX)
            gkey = s + 128 * qb + cc - WCTX + 0 * rr
            m[qb] = (band & (gkey >= 0) & (gkey < T)).astype(np.float32)
        mA = np.ascontiguousarray(m[:, :, :128].transpose(2, 0, 1)).astype(BF16)
        mB = np.ascontiguousarray(m[:, :, 128:].transpose(2, 0, 1)).astype(BF16)

        in_maps.append({
            "x": xr, "xt": xt, "mA": mA, "mB": mB,
            "wq": wq, "wk": wk, "wv": wv, "wo": wo,
            "wg1": wg1, "wg2": wg2,
            "bqh": bqh,
            "bo": bo_eff.reshape(1, D).astype(np.float32),
            "bg": bg_eff.reshape(1, D).astype(np.float32),
        })
    return in_maps, flags


def _run(inputs, trace=False):
    from concourse.bass_utils import run_bass_kernel_spmd

    in_maps, flags = _host_prep(inputs)
    if flags not in _CACHE:
        _CACHE[flags] = _build_program(flags)
    nc = _CACHE[flags]
    res = run_bass_kernel_spmd(nc, in_maps, list(range(NCORES)), trace=trace)
    out = np.zeros((B, T, D), np.float32)
    for core in range(NCORES):
        bi, ci = core // 4, core % 4
        out[bi, ci * CHUNK: (ci + 1) * CHUNK] = np.asarray(
            res.results[core]["out"], dtype=np.float32
        )
    return out, res


def kernel(**inputs):
    out, _ = _run(inputs, trace=False)
    return out
